# revision 12
# baseline (speedup 1.0000x reference)
"""CRF token-classification NLL (forward-algorithm log-partition + gold score)
on 8 Trainium2 NeuronCores, data-parallel over the batch axis.

Full inputs: emissions (1024, 512, 64) f32, transitions (64,64), start/end (64,),
tags (1024,512) int, mask (1024,512) all-ones bool.
Output: scalar f32 mean NLL.

Per-core shard: 128 batch rows. The device runs, per core:
  - the scaled linear-domain forward algorithm (matmul with exp(transitions) in
    bf16, per-step multiply by exp(emissions + C) with C = -(log 64 + 0.5) to
    keep magnitudes centered; exact per-column renormalization every 32 steps
    accumulating log-scales), producing log-partition pieces, and
  - the gold-path emission sum via one-hot (is_equal against an iota table)
    multiply-reduce over the full emissions shard.
The tiny (tag-table) pieces of the gold score -- start/end/transition lookups,
O(B*L) scalar table reads -- are folded in during unsharding.
"""

import sys

sys.path.insert(0, "/opt/trn_rl_repo")

import numpy as np

import concourse.bacc as bacc
import concourse.bass as bass  # noqa: F401  (AP helpers)
import concourse.mybir as mybir
from concourse.bass_utils import run_bass_kernel_spmd
from concourse.masks import make_identity
from concourse.tile import TileContext

P = 128          # batch shard per core (= SBUF partitions)
B = 1024
L = 512
T = 64
NCORES = 8
LC = 16          # time-steps per DMA chunk
NCH = L // LC
RENORM_PAIRS = 16   # renormalize after every 16 step-pairs (32 steps)
C_SHIFT = -(np.log(64.0) + 0.5)

f32 = mybir.dt.float32
bf16 = mybir.dt.bfloat16
i32 = mybir.dt.int32

_CACHE = {}


def _build_program():
    nc = bacc.Bacc()

    em = nc.dram_tensor("em", [P, L, T], f32, kind="ExternalInput")
    tg = nc.dram_tensor("tags_f32", [P, L], f32, kind="ExternalInput")
    tr = nc.dram_tensor("trans", [T, T], f32, kind="ExternalInput")
    st = nc.dram_tensor("start", [T, 1], f32, kind="ExternalInput")
    en = nc.dram_tensor("end", [T, 1], f32, kind="ExternalInput")
    out_emsum = nc.dram_tensor("out_emsum", [P, 1], f32, kind="ExternalOutput")
    out_c = nc.dram_tensor("out_c", [P, 1], f32, kind="ExternalOutput")
    out_lnz = nc.dram_tensor("out_lnz", [1, P], f32, kind="ExternalOutput")

    AF = mybir.ActivationFunctionType
    ALU = mybir.AluOpType
    AX = mybir.AxisListType

    with TileContext(nc) as tc:
        with (
            tc.tile_pool(name="const", bufs=1) as cp,
            tc.tile_pool(name="io", bufs=3) as iop,
            tc.tile_pool(name="work", bufs=3) as wp,
            tc.tile_pool(name="acc", bufs=2) as accp,
            tc.tile_pool(name="a_pool", bufs=4) as ap_,
            tc.tile_pool(name="ps_big", bufs=2, space="PSUM") as ps_big,
            tc.tile_pool(name="ps_small", bufs=3, space="PSUM") as ps_small,
            tc.tile_pool(name="ps_rn", bufs=1, space="PSUM") as ps_rn,
        ):
            # ---- constants ----
            id128 = cp.tile([P, P], f32, tag="id128")
            make_identity(nc, id128[:])
            id64_bf = cp.tile([T, T], bf16, tag="id64")
            make_identity(nc, id64_bf[:])

            iota_i = cp.tile([P, T], i32, tag="iota_i")
            nc.gpsimd.iota(iota_i[:], pattern=[[1, T]], channel_multiplier=0)
            iota_f = cp.tile([P, T], f32, tag="iota_f")
            nc.vector.tensor_copy(iota_f[:], iota_i[:])

            emtag_cols = cp.tile([P, L], f32, tag="emtag_cols")

            tg_sb = cp.tile([P, L], f32, tag="tg")
            nc.sync.dma_start(tg_sb[:], tg[:, :])

            tr_sb = cp.tile([T, T], f32, tag="tr")
            nc.sync.dma_start(tr_sb[:], tr[:, :])
            ee_bf = cp.tile([T, T], bf16, tag="ee")
            nc.scalar.activation(ee_bf[:], tr_sb[:], AF.Exp)

            st_sb = cp.tile([T, 1], f32, tag="st")
            nc.sync.dma_start(st_sb[:], st[:, :])
            en_sb = cp.tile([T, 1], f32, tag="en")
            nc.sync.dma_start(en_sb[:], en[:, :])
            eend_bf = cp.tile([T, 1], bf16, tag="eend")
            nc.scalar.activation(eend_bf[:], en_sb[:], AF.Exp)

            cparts = cp.tile([P, 15], f32, tag="cparts")

            cbias = cp.tile([P, 1], f32, tag="cbias")
            nc.gpsimd.memset(cbias[:], float(C_SHIFT))

            # ---- main loop over time chunks ----
            a_cur = None
            acc_prev = None
            n_renorm = 0
            for c in range(NCH):
                em_ch = iop.tile([P, LC * T], f32, tag="em_ch")
                nc.sync.dma_start(
                    em_ch[:],
                    em[:, c * LC : (c + 1) * LC, :].rearrange("p l t -> p (l t)"),
                )

                # --- numerator: per-step gold-tag emission dot products ---
                # out = (iota == tag[b,i]) * em_i ; accum_out = its row-sum
                for j in range(LC):
                    i_glob = c * LC + j
                    stt_scratch = wp.tile([P, T], f32, tag="stt")
                    nc.vector.scalar_tensor_tensor(
                        out=stt_scratch[:],
                        in0=iota_f[:],
                        scalar=tg_sb[:, i_glob : i_glob + 1],
                        in1=em_ch[:, j * T : (j + 1) * T],
                        op0=ALU.is_equal,
                        op1=ALU.mult,
                        accum_out=emtag_cols[:, i_glob : i_glob + 1],
                    )

                # --- denominator: 8 step-pairs per chunk ---
                for pc in range(LC // 2):
                    p = c * (LC // 2) + pc  # global pair index; steps 2p, 2p+1
                    tp = ps_big.tile([P, P], f32, tag="tp")
                    nc.tensor.transpose(
                        tp[:], em_ch[:, pc * 2 * T : (pc * 2 + 2) * T], id128[:]
                    )
                    expem = wp.tile([P, P], f32, tag="expem")
                    if p == 0:
                        nc.scalar.activation(
                            expem[0:T, :], tp[0:T, :], AF.Exp, bias=st_sb[:]
                        )
                        nc.scalar.activation(
                            expem[T:P, :], tp[T:P, :], AF.Exp, bias=cbias[T:P, :]
                        )
                    else:
                        nc.scalar.activation(expem[:], tp[:], AF.Exp, bias=cbias[:])

                    # even step
                    a_even = ap_.tile([T, P], bf16, tag="a")
                    if p == 0:
                        nc.vector.tensor_copy(a_even[:], expem[0:T, :])
                    else:
                        psA = ps_small.tile([T, P], f32, tag="ps_s")
                        nc.tensor.matmul(
                            out=psA[:], lhsT=ee_bf[:], rhs=a_cur[:],
                            start=True, stop=True,
                        )
                        nc.vector.tensor_tensor(
                            out=a_even[:], in0=psA[:], in1=expem[0:T, :], op=ALU.mult
                        )
                    # odd step
                    a_odd = ap_.tile([T, P], bf16, tag="a")
                    psB = ps_small.tile([T, P], f32, tag="ps_s")
                    nc.tensor.matmul(
                        out=psB[:], lhsT=ee_bf[:], rhs=a_even[:],
                        start=True, stop=True,
                    )
                    nc.vector.tensor_tensor(
                        out=a_odd[:], in0=psB[:], in1=expem[T:P, :], op=ALU.mult
                    )
                    a_cur = a_odd

                    # --- periodic exact renormalization ---
                    if (p + 1) % RENORM_PAIRS == 0 and p != (L // 2 - 1):
                        tpn = ps_rn.tile([P, T], bf16, tag="tpn")
                        nc.tensor.transpose(tpn[:], a_cur[:], id64_bf[:])
                        s_col = wp.tile([P, 1], f32, tag="s_col")
                        nc.vector.reduce_sum(out=s_col[:], in_=tpn[:], axis=AX.X)
                        r_col = wp.tile([P, 1], f32, tag="r_col")
                        nc.vector.reciprocal(r_col[:], s_col[:])
                        nc.scalar.activation(
                            cparts[:, n_renorm : n_renorm + 1], s_col[:], AF.Ln
                        )
                        n_renorm += 1
                        a_bT = wp.tile([P, T], f32, tag="a_bT")
                        nc.vector.tensor_scalar_mul(a_bT[:], tpn[:], r_col[:])
                        tpb = ps_small.tile([T, P], f32, tag="ps_s")
                        nc.tensor.transpose(tpb[:], a_bT[:], id128[:])
                        a_rn = ap_.tile([T, P], bf16, tag="a")
                        nc.vector.tensor_copy(a_rn[:], tpb[:])
                        a_cur = a_rn

            assert n_renorm == 15

            # ---- final: z = expend^T a ; outputs ----
            psZ = ps_small.tile([1, P], f32, tag="ps_s")
            nc.tensor.matmul(
                out=psZ[:], lhsT=eend_bf[:], rhs=a_cur[:], start=True, stop=True
            )
            lnz_sb = wp.tile([1, P], f32, tag="lnz")
            nc.scalar.activation(lnz_sb[:], psZ[:], AF.Ln)
            nc.sync.dma_start(out_lnz[:, :], lnz_sb[:])

            c_col = wp.tile([P, 1], f32, tag="c_col")
            nc.vector.reduce_sum(out=c_col[:], in_=cparts[:], axis=AX.X)
            nc.sync.dma_start(out_c[:, :], c_col[:])

            emsum_col = accp.tile([P, 1], f32, tag="acc")
            nc.vector.reduce_sum(out=emsum_col[:], in_=emtag_cols[:], axis=AX.X)
            nc.sync.dma_start(out_emsum[:, :], emsum_col[:])

    nc.finalize()
    return nc


def kernel(emissions, transitions, start_transitions, end_transitions, tags, mask):
    emissions = np.ascontiguousarray(np.asarray(emissions, dtype=np.float32))
    transitions = np.ascontiguousarray(np.asarray(transitions, dtype=np.float32))
    start_transitions = np.asarray(start_transitions, dtype=np.float32)
    end_transitions = np.asarray(end_transitions, dtype=np.float32)
    tags_i = np.asarray(tags)
    assert bool(np.all(np.asarray(mask))), "kernel assumes an all-ones mask"

    if "nc" not in _CACHE:
        _CACHE["nc"] = _build_program()
    nc = _CACHE["nc"]

    tags_f = tags_i.astype(np.float32)
    st2 = start_transitions.reshape(T, 1)
    en2 = end_transitions.reshape(T, 1)
    in_maps = []
    for core in range(NCORES):
        sl = slice(core * P, (core + 1) * P)
        in_maps.append(
            {
                "em": emissions[sl],
                "tags_f32": np.ascontiguousarray(tags_f[sl]),
                "trans": transitions,
                "start": st2,
                "end": en2,
            }
        )

    res = run_bass_kernel_spmd(nc, in_maps, core_ids=list(range(NCORES)))

    # host-side unshard + tiny tag-table pieces of the gold score
    tg_all = tags_i.astype(np.int64)
    s_tbl = (
        start_transitions[tg_all[:, 0]]
        + transitions[tg_all[:, :-1], tg_all[:, 1:]].sum(axis=1, dtype=np.float32)
        + end_transitions[tg_all[:, -1]]
    ).astype(np.float32)

    log_z = np.empty(B, np.float32)
    emsum = np.empty(B, np.float32)
    for core in range(NCORES):
        r = res.results[core]
        sl = slice(core * P, (core + 1) * P)
        log_z[sl] = (
            r["out_lnz"][0]
            + r["out_c"][:, 0]
            + np.float32(-(L - 1) * C_SHIFT)
        )
        emsum[sl] = r["out_emsum"][:, 0]

    loss = np.mean(log_z - (emsum + s_tbl), dtype=np.float32)
    return np.array(loss, dtype=np.float32)


if __name__ == "__main__":
    import reference as ref

    inp = {k: np.asarray(v) for k, v in ref.setup_inputs().items()}
    print("kernel:", kernel(**inp))
